# revision 1
# baseline (speedup 1.0000x reference)
"""GAT layer kernel for Trainium2, 8 NeuronCores (edge-parallel by target range).

v2: instruction-count-batched Phase B + local full-table Phase A.

Reference computes, per edge (s -> t):
    e = leaky_relu(score_tgt[t] + score_src[s]); w = exp(e)
    out[t] = sum_e w * h_proj[s] / (sum_e w + 1e-16)      (per head)

Sharding: core r owns target nodes [r*npc, (r+1)*npc).  Host groups each
core's edges into 98 windows of 128 target nodes; windows are paired into
groups of G_W for gather-call batching.  Within a window edges are grouped
by source-node quarter (quarter size 2*npc = 25088 < 32768 so the int16
dma_gather path works).

Device program (identical on all 8 cores, SPMD):
  Phase A: AllGather h_shard -> T_hin (full node features, bf16), or in
   replicate mode T_hin comes straight from the host.  Then every core
   builds the full gather table t_tab locally:
     t_tab[n] = [h_proj bf16 (128) | s_src f32 (8 slots) | s_tgt f32 (8) | pad]
   and a compact local st table shard_st[loc] = [s_tgt f32 (8 slots) | pad].
  Phase B per group of G_W windows: one dma_gather per source quarter
   (spanning all windows of the group) + one st dma_gather; one is_equal
   indicator per window; one exp/leaky pass per group; per 128-edge tile a
   single matmul accumulates [messages | denominators] into PSUM [128,132];
   divide once per node; store.
"""

import math
import numpy as np

import concourse.bass as bass
import concourse.tile as tile
from concourse import bacc, mybir
from concourse.bass_utils import run_bass_kernel_spmd
from concourse.masks import make_identity

F32 = mybir.dt.float32
BF16 = mybir.dt.bfloat16
I16 = mybir.dt.int16

N_CORES = 8
H = 4          # heads
FO = 32        # per-head out features
C = H * FO     # 128
FI = 128       # in features
RB = 256       # table row elems (bf16): [hp 128 | ss 8 | st 8 | pad] = 512B
RS = 128       # st table row elems (bf16): [st 8 | pad] = 256B
NQ = 4         # source quarters
G_W = 2        # windows per gather group
PA_CH = 4      # tiles per Phase-A chunk


class Cfg:
    def __init__(self, n_nodes, npc, Twq, b_is_zero, replicate=False):
        self.n_nodes = n_nodes
        self.npc = npc
        self.n_win = npc // 128
        self.Twq = Twq                       # [n_win, NQ]
        self.Tw = Twq.sum(axis=1)            # [n_win]
        self.b_is_zero = b_is_zero
        self.replicate = replicate
        self.n_total = npc * N_CORES
        self.QR = 2 * npc
        # group layout: groups of G_W windows; within a group tiles are laid
        # out q-major: [q0: w0 tiles, w1 tiles | q1: w0, w1 | ...]
        self.groups = []          # list of (win_list, Twg, Tgq[q], tile_base[w][q])
        goff = 0
        for g0 in range(0, self.n_win, G_W):
            ws = list(range(g0, min(g0 + G_W, self.n_win)))
            base = {}
            run = 0
            tgq = []
            for q in range(NQ):
                tgq.append(int(Twq[ws, q].sum()))
                for w in ws:
                    base[(w, q)] = goff + run
                    run += int(Twq[w, q])
            self.groups.append((ws, run, tgq, base, goff))
            goff += run
        self.tot_tiles = goff
        # per-window global tile lists (q-major inside its group)
        self.wtiles = []
        for w in range(self.n_win):
            g = self.groups[w // G_W]
            tl = []
            for q in range(NQ):
                b0 = g[3][(w, q)]
                tl += list(range(b0, b0 + int(Twq[w, q])))
            self.wtiles.append(tl)
        self.WTo = np.concatenate([[0], np.cumsum(self.Tw)[:-1]]).astype(int)

    def key(self):
        return (self.n_nodes, self.npc, self.tot_tiles, self.b_is_zero,
                self.replicate, tuple(self.Tw.tolist()),
                tuple(self.Twq.reshape(-1).tolist()))


def build_program(cfg: Cfg, repeat: int = 1, single_core: bool = False,
                  gathers_only: bool = False, st_single_packet: bool = False,
                  n_queues: int = 1):
    nc = bacc.Bacc("TRN2", target_bir_lowering=False,
                   dynamic_dma_scratch_size=65536,
                   num_swdge_queues=n_queues)
    npc, n_win, n_total = cfg.npc, cfg.n_win, cfg.n_total
    TT = cfg.tot_tiles
    ts = bass.ts

    h_shard = nc.declare_dram_parameter("h_shard", [npc, FI], BF16, isOutput=False)
    if cfg.replicate:
        h_all = nc.declare_dram_parameter("h_all", [n_total, FI], BF16,
                                          isOutput=False)
    W_p = nc.declare_dram_parameter("W", [FI, C], BF16, isOutput=False)
    brow_p = nc.declare_dram_parameter("b_row", [1, C], F32, isOutput=False)
    bcol_p = nc.declare_dram_parameter("b_col", [C, 1], BF16, isOutput=False)
    A8_p = nc.declare_dram_parameter("A8", [C, 2 * H], BF16, isOutput=False)
    iota_p = nc.declare_dram_parameter("iota", [128, 128], BF16, isOutput=False)
    qidx_p = nc.declare_dram_parameter("qidx", [128, TT * 8], I16, isOutput=False)
    tgl_p = nc.declare_dram_parameter("tgl", [128, TT], BF16, isOutput=False)
    tglt_p = nc.declare_dram_parameter("tglt", [1, TT * 128], BF16,
                                       isOutput=False)
    ciota_p = nc.declare_dram_parameter("ciota", [128, 1], F32, isOutput=False)
    out_p = nc.declare_dram_parameter("out", [npc, C], F32, isOutput=True)

    if not cfg.replicate:
        T_hin = nc.dram_tensor("T_hin", [n_total, FI], BF16, addr_space="Shared")
    t_tab = nc.dram_tensor("t_tab", [n_total, RB], BF16)

    groups8 = [list(range(N_CORES))]

    with tile.TileContext(nc) as tc:
        with tc.tile_pool(name="const", bufs=1) as const:
            iota_bf = const.tile([128, 128], BF16)
            nc.sync.dma_start(out=iota_bf[:], in_=iota_p[:, :])
            TGL = const.tile([128, TT], BF16)
            nc.sync.dma_start(out=TGL[:], in_=tgl_p[:, :])
            ident = const.tile([128, 128], F32)
            make_identity(nc, ident[:])
            ident_bf = const.tile([128, 128], BF16)
            make_identity(nc, ident_bf[:])
            W_aug = const.tile([FI, C + 2 * H], BF16)
            Bb = const.tile([128, C + 2 * H], F32)
            STW = const.tile([128, n_win * H], BF16)
            CIOTA = const.tile([128, 1], F32)
            nc.sync.dma_start(out=CIOTA[:], in_=ciota_p[:, :])
            ones_row = const.tile([1, 128], BF16)
            nc.vector.memset(ones_row[:], 1.0)

            # ---- distribute node features (issued first; setup and
            # the mini Phase A below overlap with the collective) ----
            if not cfg.replicate:
                h_loc = nc.dram_tensor("h_loc", [npc, FI], BF16)
                nc.gpsimd.dma_start(out=h_loc[:, :], in_=h_shard[:, :])
                tc.strict_bb_all_engine_barrier()
                if single_core:
                    nc.gpsimd.dma_start(out=T_hin[0:npc, :], in_=h_loc[:, :])
                else:
                    nc.gpsimd.collective_compute(
                        "AllGather", mybir.AluOpType.bypass,
                        replica_groups=groups8,
                        ins=[h_loc[:, :]], outs=[T_hin[:, :]])
                h_src = T_hin
            else:
                h_src = h_all

            setup = tc.alloc_tile_pool(name="setup", bufs=1)
            with tc.tile_pool(name="psetup", bufs=1, space="PSUM") as psetup:
                # NOTE: pad columns of t_tab/shard_st rows are never read by
                # Phase B (only cols 0:144 / 0:16 of gathered rows are used),
                # so they are left uninitialized.

                # W_aug = [W bf16 | (A8^T W^T)^T scores], Bb = bias row bcast
                W_sb = setup.tile([FI, C], BF16)
                nc.sync.dma_start(out=W_sb[:], in_=W_p[:, :])
                A8_bf = setup.tile([C, 2 * H], BF16)
                nc.sync.dma_start(out=A8_bf[:], in_=A8_p[:, :])
                psWt = psetup.tile([C, FI], BF16)
                nc.tensor.transpose(psWt[:], W_sb[:], ident_bf[:])
                rhs129 = setup.tile([C, FI + 1], BF16)
                nc.vector.tensor_copy(rhs129[:, 0:FI], psWt[:])
                nc.sync.dma_start(out=rhs129[:, FI:FI + 1], in_=bcol_p[:, :])

                ps8 = psetup.tile([2 * H, FI + 1], F32)
                nc.tensor.matmul(ps8[:], A8_bf[:], rhs129[:], start=True, stop=True)

                S8 = setup.tile([2 * H, FI], F32)
                nc.vector.tensor_copy(S8[:], ps8[:, 0:FI])
                psT = psetup.tile([FI, 2 * H], F32)
                nc.tensor.transpose(psT[:], S8[:], ident[:2 * H, :2 * H])
                nc.vector.tensor_copy(W_aug[:, C:C + 2 * H], psT[:])

                C8 = setup.tile([2 * H, 1], F32)
                nc.vector.tensor_copy(C8[:], ps8[:, FI:FI + 1])
                psC = psetup.tile([1, 2 * H], F32)
                nc.tensor.transpose(psC[:], C8[:], ident[:2 * H, :2 * H])

                nc.vector.tensor_copy(W_aug[:, 0:C], W_sb[:])

                brow136 = setup.tile([1, C + 2 * H], F32)
                nc.sync.dma_start(out=brow136[:, 0:C], in_=brow_p[:, :])
                nc.vector.tensor_copy(brow136[:, C:C + 2 * H], psC[:])
                ones = setup.tile([1, 128], F32)
                nc.vector.memset(ones[:], 1.0)
                psBB = psetup.tile([128, C + 2 * H], F32)
                nc.tensor.matmul(psBB[:], ones[:], brow136[:], start=True, stop=True)
                nc.vector.tensor_copy(Bb[:], psBB[:])
            setup.release()

            # ---- Phase A-st: local st table from h_shard ----
            pa = tc.alloc_tile_pool(name="pa", bufs=3)
            with tc.tile_pool(name="psa", bufs=2, space="PSUM") as psa:
                for j in range(npc // (128 * 2)):
                    Xt = pa.tile([128, 256], BF16, tag="Xst")
                    nc.sync.dma_start_transpose(Xt[:], h_shard[ts(j, 256), :])
                    psS = psa.tile([128, 2 * 2 * H], F32)
                    for k in range(2):
                        nc.tensor.matmul(psS[:, k * 2 * H:(k + 1) * 2 * H],
                                         Xt[:, k * 128:(k + 1) * 128],
                                         W_aug[:, C:C + 2 * H],
                                         start=True, stop=True)
                    # keep only s_tgt (cols H:2H of each score block), write
                    # straight into the SBUF-resident per-window st table
                    sc3 = STW[:, j * 2 * H:(j + 1) * 2 * H].rearrange(
                        "p (t h) -> p t h", h=H)
                    ps3 = psS[:].rearrange("p (t h) -> p t h", h=2 * H)
                    nc.vector.tensor_tensor(
                        out=sc3, in0=ps3[:, :, H:2 * H],
                        in1=Bb[:, C + H:C + 2 * H].unsqueeze(1).to_broadcast(
                            [128, 2, H]),
                        op=mybir.AluOpType.add)

                if not cfg.replicate:
                    tc.strict_bb_all_engine_barrier()
                # ---- Phase A-big: full table from h_src ----
                # Table rows are block-interleaved: node n = j*512 + k*128 + p
                # lands at table row j*512 + p*4 + k, so each partition stores
                # 4 consecutive full 512B rows as ONE contiguous 2KB
                # descriptor (qidx on the host uses the same permutation).
                for j in range(n_total // (128 * PA_CH)):
                    Xt = pa.tile([128, 128 * PA_CH], BF16, tag="Xbig")
                    nc.sync.dma_start_transpose(
                        Xt[:], h_src[ts(j, 128 * PA_CH), :])
                    ROW = pa.tile([128, PA_CH * RB], BF16, tag="ROW")
                    rv = ROW[:].rearrange("p (i c) -> p i c", c=RB)
                    for k in range(0, PA_CH, 2):
                        psA = psa.tile([128, 2 * (C + 2 * H)], F32)
                        for i in range(2):
                            nc.tensor.matmul(
                                psA[:, i * (C + 2 * H):(i + 1) * (C + 2 * H)],
                                Xt[:, (k + i) * 128:(k + i + 1) * 128],
                                W_aug[:], start=True, stop=True)
                        pv = psA[:].rearrange("p (t c) -> p t c", c=C + 2 * H)
                        nc.vector.tensor_tensor(
                            out=rv[:, k:k + 2, 0:C], in0=pv[:, :, 0:C],
                            in1=Bb[:, 0:C].unsqueeze(1).to_broadcast(
                                [128, 2, C]),
                            op=mybir.AluOpType.add)
                        nc.vector.tensor_tensor(
                            out=rv[:, k:k + 2, C:C + 4 * H].bitcast(F32),
                            in0=pv[:, :, C:C + 2 * H],
                            in1=Bb[:, C:C + 2 * H].unsqueeze(1).to_broadcast(
                                [128, 2, 2 * H]),
                            op=mybir.AluOpType.add)
                    nc.sync.dma_start(
                        out=t_tab[ts(j, 128 * PA_CH), :].rearrange(
                            "(p i) c -> p (i c)", i=PA_CH),
                        in_=ROW[:])
            pa.release()
            tc.strict_bb_all_engine_barrier()

            # ---------------- Phase B ----------------
            if repeat == 0:
                zo = tc.alloc_tile_pool(name="zo", bufs=1)
                Z = zo.tile([128, C], F32)
                nc.vector.memset(Z[:], 0.0)
                for w in range(n_win):
                    nc.sync.dma_start(out=out_p[ts(w, 128), :], in_=Z[:])
                zo.release()
                repeat_range = range(0)
            else:
                repeat_range = range(repeat)

            pb = tc.alloc_tile_pool(name="pb", bufs=3)
            pg = tc.alloc_tile_pool(name="pg", bufs=2)
            pr = tc.alloc_tile_pool(name="pr", bufs=2)
            pi = tc.alloc_tile_pool(name="pi", bufs=3)
            with tc.tile_pool(name="psm", bufs=2, space="PSUM") as psm, \
                 tc.tile_pool(name="pst", bufs=2, space="PSUM") as pst, \
                 tc.tile_pool(name="ptg", bufs=2, space="PSUM") as ptg:
              for _rep in repeat_range:
                for (ws, Twg, Tgq, tbase, goff) in cfg.groups:
                    QI = pb.tile([128, Twg * 8], I16, tag="QI")
                    nc.sync.dma_start(out=QI[:],
                                      in_=qidx_p[:, goff * 8:(goff + Twg) * 8])
                    TGT = pb.tile([1, Twg * 128], BF16, tag="TGT")
                    nc.sync.dma_start(
                        out=TGT[:],
                        in_=tglt_p[0:1, goff * 128:(goff + Twg) * 128])

                    G = pg.tile([128, Twg * RB], BF16, tag="G")
                    off = 0
                    for q in range(NQ):
                        Tq = Tgq[q]
                        if Tq == 0:
                            continue
                        dst = G[:, off * RB:(off + Tq) * RB].rearrange(
                            "p (t c) -> p t c", c=RB)
                        nc.gpsimd.dma_gather(
                            dst, t_tab[q * cfg.QR:(q + 1) * cfg.QR, :],
                            QI[:, off * 8:(off + Tq) * 8],
                            Tq * 128, Tq * 128, RB, single_packet=False,
                            queue_num=q % n_queues)
                        off += Tq
                    if gathers_only:
                        continue

                    # per-edge s_tgt via transposed one-hot indicators:
                    #   psT = ones ⊗ tglT (row-replicate), INDT = (psT == p),
                    #   st_e = INDT^T-contraction @ STW[window]
                    ps_st = pst.tile([128, Twg * H], F32)
                    for w in ws:
                        Tw = int(cfg.Tw[w])
                        wrel = int(cfg.WTo[w]) - goff
                        tl = cfg.wtiles[w]
                        for kk in range(0, Tw, 4):
                            m = min(4, Tw - kk)
                            psT = ptg.tile([128, 512], F32)
                            nc.tensor.matmul(
                                psT[:, 0:m * 128], ones_row[:],
                                TGT[0:1, (wrel + kk) * 128:
                                    (wrel + kk + m) * 128],
                                start=True, stop=True)
                            INDT = pi.tile([128, 512], BF16, tag="INDT")
                            nc.vector.tensor_scalar(
                                INDT[:, 0:m * 128], psT[:, 0:m * 128],
                                CIOTA[:], None, mybir.AluOpType.is_equal)
                            for i in range(m):
                                lt = tl[kk + i] - goff
                                nc.tensor.matmul(
                                    ps_st[:, lt * H:(lt + 1) * H],
                                    INDT[:, i * 128:(i + 1) * 128],
                                    STW[:, w * H:(w + 1) * H],
                                    start=True, stop=True)

                    ssv = G[:].rearrange("p (t c) -> p t c", c=RB)[
                        :, :, C:C + 2 * H].bitcast(F32)
                    E = pb.tile([128, Twg * H], F32, tag="E")
                    nc.vector.tensor_tensor(
                        out=E[:].rearrange("p (t h) -> p t h", h=H),
                        in0=ssv,
                        in1=ps_st[:].rearrange("p (t h) -> p t h", h=H),
                        op=mybir.AluOpType.add)
                    E2 = pb.tile([128, Twg * H], F32, tag="E2")
                    nc.vector.scalar_tensor_tensor(
                        E2[:], E[:], 0.2, E[:],
                        op0=mybir.AluOpType.mult, op1=mybir.AluOpType.max)
                    R = pr.tile([128, Twg * 132], BF16, tag="R")
                    r3 = R[:].rearrange("p (t c) -> p t c", c=132)
                    nc.scalar.activation(
                        r3[:, :, 128:132],
                        E2[:].rearrange("p (t h) -> p t h", h=H),
                        mybir.ActivationFunctionType.Exp)
                    g4 = G[:].rearrange("p (t c) -> p t c", c=RB)[
                        :, :, 0:C].rearrange("p t (h f) -> p t h f", h=H)
                    m4 = r3[:, :, 0:C].rearrange("p t (h f) -> p t h f", h=H)
                    wb4 = r3[:, :, 128:132].unsqueeze(3).to_broadcast(
                        [128, Twg, H, FO])
                    nc.vector.tensor_tensor(out=m4, in0=g4, in1=wb4,
                                            op=mybir.AluOpType.mult)

                    for w in ws:
                        Tw = int(cfg.Tw[w])
                        IND = pi.tile([128, Tw * 128], BF16, tag="IND")
                        i3 = IND[:].rearrange("p (t c) -> p t c", c=128)
                        wto = int(cfg.WTo[w])
                        nc.vector.tensor_tensor(
                            out=i3,
                            in0=iota_bf[:].unsqueeze(1).to_broadcast(
                                [128, Tw, 128]),
                            in1=TGL[:, wto:wto + Tw].unsqueeze(2).to_broadcast(
                                [128, Tw, 128]),
                            op=mybir.AluOpType.is_equal)
                        ps = psm.tile([128, 132], F32)
                        tl = cfg.wtiles[w]
                        for k, gt in enumerate(tl):
                            lt = gt - goff
                            nc.tensor.matmul(
                                ps[:], IND[:, k * 128:(k + 1) * 128],
                                R[:, lt * 132:(lt + 1) * 132],
                                start=(k == 0), stop=(k == len(tl) - 1))
                        DEN = pb.tile([128, H], F32, tag="DEN")
                        nc.vector.tensor_scalar(
                            DEN[:], ps[:, 128:132], 1e-16, None,
                            mybir.AluOpType.add)
                        RCP = pb.tile([128, H], F32, tag="RCP")
                        nc.vector.reciprocal(RCP[:], DEN[:])
                        O = pb.tile([128, C], F32, tag="O")
                        o3 = O[:].rearrange("p (h f) -> p h f", h=H)
                        nc.vector.tensor_tensor(
                            out=o3,
                            in0=ps[:, 0:C].rearrange("p (h f) -> p h f", h=H),
                            in1=RCP[:].unsqueeze(2).to_broadcast([128, H, FO]),
                            op=mybir.AluOpType.mult)
                        if not cfg.b_is_zero:
                            nc.vector.tensor_tensor(
                                out=O[:], in0=O[:], in1=Bb[:, 0:C],
                                op=mybir.AluOpType.add)
                        nc.sync.dma_start(out=out_p[ts(w, 128), :], in_=O[:])

            for _pool in (pi, pr, pg, pb):
                _pool.release()

    if not nc.is_finalized():
        nc.finalize()
    return nc


# ---------------------------------------------------------------------------
# host side
# ---------------------------------------------------------------------------

def _wrap16(vals, n_slots):
    """Slot s -> [s % 16, s // 16], replicated across the 8 Q7 groups."""
    a = np.zeros((16, n_slots // 16), dtype=np.int16)
    a[np.arange(len(vals)) % 16, np.arange(len(vals)) // 16] = vals
    return np.tile(a, (8, 1))


def compute_cfg(edge_index, n_nodes, b, replicate=False):
    npc = int(math.ceil(n_nodes / (N_CORES * 128))) * 128
    n_win = npc // 128
    QR = 2 * npc
    src = np.asarray(edge_index[0], dtype=np.int64)
    tgt = np.asarray(edge_index[1], dtype=np.int64)
    core = tgt // npc
    loc = tgt - core * npc
    w = loc >> 7
    q = src // QR
    flat = (core * n_win + w) * NQ + q
    counts = np.bincount(flat, minlength=N_CORES * n_win * NQ).reshape(
        N_CORES, n_win, NQ)
    mx = counts.max(axis=0)
    Twq = np.ceil(mx / 128).astype(np.int64)
    b_is_zero = bool(np.all(np.asarray(b) == 0.0))
    return Cfg(n_nodes, npc, Twq, b_is_zero, replicate=replicate)


def prep_inputs(h_in, edge_index, W, b, a_src, a_tgt, cfg: Cfg):
    import ml_dtypes
    npc, n_win, QR = cfg.npc, cfg.n_win, cfg.QR
    TT = cfg.tot_tiles
    src = np.asarray(edge_index[0], dtype=np.int64)
    tgt = np.asarray(edge_index[1], dtype=np.int64)

    h_in = np.asarray(h_in, dtype=np.float32)
    W = np.asarray(W, dtype=np.float32)
    b = np.asarray(b, dtype=np.float32).reshape(-1)
    a_src = np.asarray(a_src, dtype=np.float32)
    a_tgt = np.asarray(a_tgt, dtype=np.float32)

    A8 = np.zeros((C, 2 * H), dtype=np.float32)
    for h in range(H):
        A8[h * FO:(h + 1) * FO, h] = a_src[h]
        A8[h * FO:(h + 1) * FO, H + h] = a_tgt[h]
    iota = np.tile(np.arange(128, dtype=np.float32), (128, 1)).astype(
        ml_dtypes.bfloat16)

    h_pad = np.zeros((cfg.n_total, FI), dtype=np.float32)
    h_pad[:cfg.n_nodes] = h_in
    h_bf = h_pad.astype(ml_dtypes.bfloat16)

    # global slot base per (w, q) from the group layout
    slot_base = np.zeros((n_win, NQ), dtype=np.int64)
    for (ws, Twg, Tgq, tbase, goff) in cfg.groups:
        for (w, q), tb in tbase.items():
            slot_base[w, q] = tb * 128

    core = tgt // npc
    in_maps = []
    for r in range(N_CORES):
        m = core == r
        s_r = src[m]
        loc = tgt[m] - r * npc
        w_r = loc >> 7
        q_r = s_r // QR
        key = w_r * NQ + q_r
        order = np.argsort(key, kind="stable")
        s_r, loc, w_r, q_r, key = (a[order] for a in (s_r, loc, w_r, q_r, key))
        cnt = np.bincount(key, minlength=n_win * NQ)
        starts = np.concatenate([[0], np.cumsum(cnt)[:-1]])
        pos = np.arange(len(key)) - np.repeat(starts, cnt)
        slot = slot_base[w_r, q_r] + pos

        qidx_flat = np.zeros(TT * 128, dtype=np.int16)
        tgl_flat = np.full(TT * 128, -1.0, dtype=np.float32)
        # block-interleaved table row (see Phase A-big store layout)
        s_row = (s_r // 512) * 512 + (s_r % 128) * 4 + (s_r // 128) % 4
        qidx_flat[slot] = (s_row - q_r * QR).astype(np.int16)
        tgl_flat[slot] = (loc & 127).astype(np.float32)

        qidx = np.zeros((128, TT * 8), dtype=np.int16)
        for (ws, Twg, Tgq, tbase, goff) in cfg.groups:
            off = goff
            for q in range(NQ):
                Tq = Tgq[q]
                if Tq == 0:
                    continue
                vals = qidx_flat[off * 128:(off + Tq) * 128]
                qidx[:, off * 8:(off + Tq) * 8] = _wrap16(vals, Tq * 128)
                off += Tq

        # tgl in window-major tile order (both layouts)
        worder = np.concatenate([cfg.wtiles[w] for w in range(n_win)])
        tgl_wm = tgl_flat.reshape(TT, 128)[worder]      # [wm tile, 128]
        tgl = tgl_wm.T.astype(ml_dtypes.bfloat16)       # [128, wm tile]
        tglt = np.ascontiguousarray(
            tgl_wm.reshape(1, TT * 128)).astype(ml_dtypes.bfloat16)

        im = {
            "h_shard": np.ascontiguousarray(h_bf[r * npc:(r + 1) * npc]),
            "W": W.astype(ml_dtypes.bfloat16),
            "b_row": b.reshape(1, C),
            "b_col": b.reshape(C, 1).astype(ml_dtypes.bfloat16),
            "A8": A8.astype(ml_dtypes.bfloat16),
            "iota": iota,
            "qidx": qidx,
            "tgl": np.ascontiguousarray(tgl),
            "tglt": tglt,
            "ciota": np.arange(128, dtype=np.float32).reshape(128, 1),
        }
        if cfg.replicate:
            im["h_all"] = h_bf
        in_maps.append(im)
    return in_maps


_prog_cache = {}


def kernel(h_in, edge_index, W, b, a_src, a_tgt):
    n_nodes = h_in.shape[0]
    cfg = compute_cfg(edge_index, n_nodes, b)
    key = cfg.key()
    if key not in _prog_cache:
        _prog_cache[key] = build_program(cfg, n_queues=4)
    nc = _prog_cache[key]
    in_maps = prep_inputs(h_in, edge_index, W, b, a_src, a_tgt, cfg)
    res = run_bass_kernel_spmd(nc, in_maps, list(range(N_CORES)))
    out = np.concatenate([res.results[r]["out"] for r in range(N_CORES)], axis=0)
    return np.ascontiguousarray(out[:n_nodes])

